# revision 28
# baseline (speedup 1.0000x reference)
"""GCNConvRnd kernel for 8 Trainium2 NeuronCores (Bass/Tile).

out = segment_sum((x @ W.T)[src[keep]] * ew[keep], dst[keep], N) + bias

Strategy (dst-sharded, W applied after aggregation):
  * nodes / output sharded 12500 per core; kept edges partitioned by dst shard
  * full x replicated to every core; each core gathers x[src] rows for its own
    edges with the GPSIMD dma_gather custom instruction (int16 indices), so
    src is split into 4 chunks of 25000 rows (chunk = src // 25000, local
    index = src - chunk*25000 fits int16)
  * each core's edges are sorted by dst and packed into "windows" of <=128
    consecutive dst nodes where every (window, chunk) pair holds <= Q*128
    edges; each (window, chunk) is padded to exactly Q blocks of 128 edges
    -> fully static, SPMD-uniform program (B = 4*Q blocks per window)
  * gather streams are chunk-major (all blocks of chunk m contiguous), so a
    handful of large dma_gather calls per chunk move all rows; compute
    consumes blocks window-major at statically known positions
  * per 128-edge block: S_T[e, d] = (iota[e,d] == dstv[e]) * ew[e]  (one DVE
    tensor_scalar), then PE matmul  psum[f, d] += G[e, f].T @ S_T[e, d]
    accumulating over the window's 4*Q blocks
  * per window: PSUM -> SBUF accumulator column w
  * epilogue: out2 = W @ acc (+bias) in 512-wide chunks, DMA to HBM
  * host unpacks windows back to node order
"""

import os
import numpy as np
from contextlib import ExitStack

import concourse.bass as bass
from concourse.bass import AP
import concourse.mybir as mybir
import concourse.tile as tile
from concourse import bacc
from concourse.bass_utils import run_bass_kernel_spmd

N_NODES = 100000
F = 128
P = 128
NC = 8
NPC = N_NODES // NC      # 12500 nodes per core
NCHUNKS = 4
CHUNK = N_NODES // NCHUNKS  # 25000 rows per src chunk (int16-addressable)

# Tunables
Q = int(os.environ.get("GCN_Q", "3"))        # blocks per (window, chunk)
NB = int(os.environ.get("GCN_NB", "16"))    # class-stream blocks per gather call
G_BUFS = int(os.environ.get("GCN_GBUFS", "4"))
S_BUFS = int(os.environ.get("GCN_SBUFS", "6"))
PS_BUFS = int(os.environ.get("GCN_PSBUFS", "4"))
REPS = int(os.environ.get("GCN_REPS", "1"))  # in-NEFF repetitions (timing only)
GDT = os.environ.get("GCN_GDT", "bf16")      # gather/matmul dtype: bf16 | f32
ABL = os.environ.get("GCN_ABL", "none")      # ablation: none|nosgen|nomm|gonly|conly

f32 = mybir.dt.float32
i16 = mybir.dt.int16
bf16 = mybir.dt.bfloat16
g_dt = bf16 if GDT == "bf16" else f32

_PROGRAM_CACHE: dict = {}


def _preprocess(edge_src, edge_dst, edge_weight, idx_keep, q):
    """Shard kept edges by dst, pack dst windows under per-chunk quotas, and
    emit the static device layout.

    Returns None if quotas are infeasible (a single node overflows a chunk
    quota); caller bumps q.
    """
    src = np.ascontiguousarray(edge_src)[idx_keep].astype(np.int64)
    dst = np.ascontiguousarray(edge_dst)[idx_keep].astype(np.int64)
    ew = np.ascontiguousarray(edge_weight)[idx_keep].astype(np.float32)
    order = np.argsort(dst, kind="stable")
    src, dst, ew = src[order], dst[order], ew[order]
    core_bounds = np.searchsorted(dst, np.arange(NC + 1) * NPC)

    QCAP = q * P
    B = NCHUNKS * q  # compute blocks per window
    percore = []
    for c in range(NC):
        lo, hi = int(core_bounds[c]), int(core_bounds[c + 1])
        dl = dst[lo:hi] - c * NPC
        ch = src[lo:hi] // CHUNK
        # counts per (node, chunk)
        cnts = np.bincount(dl * NCHUNKS + ch, minlength=NPC * NCHUNKS).reshape(
            NPC, NCHUNKS
        )
        # greedy windows under 128-node span + per-chunk quota
        wins = []
        n = 0
        cl = cnts.tolist()
        while n < NPC:
            s = n
            acc = [0, 0, 0, 0]
            while n < NPC and (n - s) < P:
                row = cl[n]
                if any(acc[m] + row[m] > QCAP for m in range(NCHUNKS)):
                    break
                for m in range(NCHUNKS):
                    acc[m] += row[m]
                n += 1
            if n == s:
                return None
            wins.append((s, n))
        percore.append((lo, hi, dl, ch, cnts, wins))

    W_CAP = max(len(pc[5]) for pc in percore)
    W_CAP = -(-W_CAP // 4) * 4  # multiple of 4 -> epilogue chunks of 512
    NBLK = W_CAP * B             # compute blocks per core
    CSB = W_CAP * q              # class-stream blocks per chunk per core
    NIDX = CSB * P               # idxs per chunk stream

    idxbuf = np.zeros((NC, P, NCHUNKS * (NIDX // 16)), np.int16)
    dstv = np.zeros((NC, NBLK * P), np.float32)
    ewv = np.zeros((NC, NBLK * P), np.float32)
    metas = []
    for c, (lo, hi, dl, ch, cnts, wins) in enumerate(percore):
        ne = hi - lo
        # window id / window start node per node
        win_of = np.zeros(NPC, np.int64)
        wstart = np.zeros(NPC, np.int64)
        for w, (s, e) in enumerate(wins):
            win_of[s:e] = w
            wstart[s:e] = s
        src_l0 = src[lo:hi] - ch * CHUNK
        wj0 = win_of[dl]
        # sort core edges by (window, chunk, src): ascending src within each
        # (window, chunk) gather segment maximizes DRAM row-buffer hits
        o2 = np.lexsort((src_l0, ch, wj0))
        src_l = src_l0[o2]
        ew_l = ew[lo:hi][o2]
        dl_l = dl[o2]
        ch_l = ch[o2]
        wj = wj0[o2]
        swj = wstart[dl_l]
        # rank within (window, chunk) segment
        seg = wj * NCHUNKS + ch_l
        segcnt = np.bincount(seg, minlength=len(wins) * NCHUNKS)
        segstart = np.zeros(len(wins) * NCHUNKS + 1, np.int64)
        np.cumsum(segcnt, out=segstart[1:])
        rank = np.arange(ne) - segstart[seg]
        # slot within chunk stream: window w owns stream blocks [w*q,(w+1)*q)
        slot = wj * (q * P) + rank
        # fill idx buffer: chunk stream m, idx i -> partition i%16, col i//16
        cols = slot // 16
        parts = slot % 16
        base_cols = ch_l * (NIDX // 16)
        flat16 = np.zeros((16, NCHUNKS * (NIDX // 16)), np.int16)
        flat16[parts, base_cols + cols] = src_l.astype(np.int16)
        idxbuf[c] = np.tile(flat16, (8, 1))
        # compute-order block arrays
        cb = slot // P          # class block = w*q + qq
        pp = slot % P
        qq = cb % q
        blk = wj * B + ch_l * q + qq
        dv = np.zeros((NBLK, P), np.float32)
        ev = np.zeros((NBLK, P), np.float32)
        dv[blk, pp] = (dl_l - swj).astype(np.float32)
        ev[blk, pp] = ew_l
        dstv[c] = dv.reshape(-1)
        ewv[c] = ev.reshape(-1)
        metas.append(wins)

    dstv = np.ascontiguousarray(
        dstv.reshape(NC, NBLK, P).transpose(0, 2, 1)
    )
    ewv = np.ascontiguousarray(ewv.reshape(NC, NBLK, P).transpose(0, 2, 1))
    return idxbuf, dstv, ewv, metas, W_CAP, NBLK, CSB


def _build_program(W_CAP, q, NBLK, CSB):
    key = (W_CAP, q, NBLK, CSB, NB, G_BUFS, S_BUFS, PS_BUFS, REPS, GDT, ABL)
    if key in _PROGRAM_CACHE:
        return _PROGRAM_CACHE[key]

    B = NCHUNKS * q
    NIDX = CSB * P
    IDXCOLS = NCHUNKS * (NIDX // 16)

    nc = bacc.Bacc(
        "TRN2",
        target_bir_lowering=False,
        debug=False,
        enable_asserts=False,
        num_devices=NC,
        num_swdge_queues=4,
    )
    x_h = nc.dram_tensor("x", [N_NODES, F], g_dt, kind="ExternalInput")
    idx_d = nc.dram_tensor("idx", [P, IDXCOLS], i16, kind="ExternalInput").ap()
    dstv_d = nc.dram_tensor("dstv", [P, NBLK], g_dt, kind="ExternalInput").ap()
    ewv_d = nc.dram_tensor("ewv", [P, NBLK], g_dt, kind="ExternalInput").ap()
    wt_d = nc.dram_tensor("wt", [P, P], f32, kind="ExternalInput").ap()  # W.T
    bias_d = nc.dram_tensor("biasv", [P, 1], f32, kind="ExternalInput").ap()
    out_d = nc.dram_tensor("out", [P, W_CAP * P], f32, kind="ExternalOutput").ap()

    if GDT == "bf16":
        import ml_dtypes
        iota_np = np.broadcast_to(
            np.arange(P, dtype=np.float32).astype(ml_dtypes.bfloat16), (P, P)
        ).copy()
    else:
        iota_np = np.broadcast_to(np.arange(P, dtype=np.float32), (P, P)).copy()
    iota_d = nc.inline_tensor(iota_np, "iota").ap()

    NOCHUNK = (W_CAP * P) // 512
    NGC = (CSB + NB - 1) // NB  # gather calls per chunk

    with tile.TileContext(nc) as tc, ExitStack() as ctx:
        const = ctx.enter_context(tc.tile_pool(name="const", bufs=1))
        gpools = [
            ctx.enter_context(tc.tile_pool(name=f"g{m}", bufs=G_BUFS))
            for m in range(NCHUNKS)
        ]
        spool = ctx.enter_context(tc.tile_pool(name="s", bufs=S_BUFS))
        pspool = ctx.enter_context(tc.tile_pool(name="ps", bufs=PS_BUFS, space="PSUM"))
        ps2pool = ctx.enter_context(tc.tile_pool(name="ps2", bufs=2, space="PSUM"))
        stpool = ctx.enter_context(tc.tile_pool(name="st", bufs=2))

        iota_sb = const.tile([P, P], g_dt)
        nc.sync.dma_start(out=iota_sb[:], in_=iota_d[:])
        wt_sb = const.tile([P, P], f32)
        nc.sync.dma_start(out=wt_sb[:], in_=wt_d[:])
        bias_sb = const.tile([P, 1], f32)
        nc.sync.dma_start(out=bias_sb[:], in_=bias_d[:])
        idx_sb = const.tile([P, IDXCOLS], i16)
        nc.sync.dma_start(out=idx_sb[:], in_=idx_d[:])
        dstv_sb = const.tile([P, NBLK], g_dt)
        nc.sync.dma_start(out=dstv_sb[:], in_=dstv_d[:])
        ewv_sb = const.tile([P, NBLK], g_dt)
        nc.sync.dma_start(out=ewv_sb[:], in_=ewv_d[:])
        acc = const.tile([P, W_CAP * P], f32)

        g_tiles = {}

        ones_sb = const.tile([P, P], g_dt)
        nc.vector.memset(ones_sb[:], 1.0)

        def body():
            g_tiles.clear()
            if ABL == "sgonly":
                for w in range(W_CAP):
                    sg = spool.tile([P, B, P], g_dt)
                    nc.vector.tensor_tensor(
                        out=sg[:, :, :],
                        in0=dstv_sb[:, w * B:(w + 1) * B]
                        .unsqueeze(2).broadcast_to([P, B, P]),
                        in1=iota_sb[:, :].unsqueeze(1).broadcast_to([P, B, P]),
                        op=mybir.AluOpType.is_equal,
                    )
                    nc.vector.tensor_tensor(
                        out=sg[:, :, :],
                        in0=sg[:, :, :],
                        in1=ewv_sb[:, w * B:(w + 1) * B]
                        .unsqueeze(2).broadcast_to([P, B, P]),
                        op=mybir.AluOpType.mult,
                    )
                    ps = pspool.tile([P, P], f32, space="PSUM")
                    nc.tensor.matmul(out=ps[:], lhsT=ones_sb[:],
                                     rhs=sg[:, 0, :], start=True, stop=True)
                return
            if ABL == "both":
                # gathers + S-gen, no cross-dependencies: tests pure
                # DVE vs DMA hardware interference
                for w in range(W_CAP):
                    sg = spool.tile([P, B, P], g_dt)
                    nc.vector.tensor_tensor(
                        out=sg[:, :, :],
                        in0=dstv_sb[:, w * B:(w + 1) * B]
                        .unsqueeze(2).broadcast_to([P, B, P]),
                        in1=iota_sb[:, :].unsqueeze(1).broadcast_to([P, B, P]),
                        op=mybir.AluOpType.is_equal,
                    )
                    nc.vector.tensor_tensor(
                        out=sg[:, :, :],
                        in0=sg[:, :, :],
                        in1=ewv_sb[:, w * B:(w + 1) * B]
                        .unsqueeze(2).broadcast_to([P, B, P]),
                        op=mybir.AluOpType.mult,
                    )
                    ps = pspool.tile([P, P], f32, space="PSUM")
                    nc.tensor.matmul(out=ps[:], lhsT=ones_sb[:],
                                     rhs=sg[:, 0, :], start=True, stop=True)
                    for m in range(NCHUNKS):
                        for qq in range(q):
                            ensure_gather(m, divmod(w * q + qq, NB)[0])
                return
            if ABL == "mmonly":
                for w in range(W_CAP):
                    ps = pspool.tile([P, P], f32, space="PSUM")
                    for m in range(NCHUNKS):
                        for qq in range(q):
                            first = m == 0 and qq == 0
                            last = m == NCHUNKS - 1 and qq == q - 1
                            nc.tensor.matmul(
                                out=ps[:], lhsT=ones_sb[:], rhs=ones_sb[:],
                                start=first, stop=last)
                    nc.scalar.copy(out=acc[:, w * P:(w + 1) * P], in_=ps[:])
                return
            for w in range(W_CAP):
                # grouped S generation: one is_equal + one mult over the
                # window's B blocks (broadcast APs), instead of B narrow ops
                if ABL in ("none", "nomm"):
                    sg = spool.tile([P, B, P], g_dt)
                    nc.vector.tensor_tensor(
                        out=sg[:, :, :],
                        in0=dstv_sb[:, w * B:(w + 1) * B]
                        .unsqueeze(2).broadcast_to([P, B, P]),
                        in1=iota_sb[:, :].unsqueeze(1).broadcast_to([P, B, P]),
                        op=mybir.AluOpType.is_equal,
                    )
                    nc.vector.tensor_tensor(
                        out=sg[:, :, :],
                        in0=sg[:, :, :],
                        in1=ewv_sb[:, w * B:(w + 1) * B]
                        .unsqueeze(2).broadcast_to([P, B, P]),
                        op=mybir.AluOpType.mult,
                    )
                if ABL in ("gonly", "nosgen", "nomm"):
                    if ABL == "nomm":
                        for m in range(NCHUNKS):
                            for qq in range(q):
                                ensure_gather(m, divmod(w * q + qq, NB)[0])
                        continue
                    ps = pspool.tile([P, P], f32, space="PSUM")
                    for m in range(NCHUNKS):
                        for qq in range(q):
                            cb = w * q + qq
                            t, col = divmod(cb, NB)
                            if ABL == "gonly":
                                g = ensure_gather(m, t)
                                if m == 0 and qq == 0:
                                    nc.tensor.matmul(
                                        out=ps[:], lhsT=ones_sb[:],
                                        rhs=ones_sb[:], start=True, stop=True)
                                continue
                            g = ensure_gather(m, t)
                            first = m == 0 and qq == 0
                            last = m == NCHUNKS - 1 and qq == q - 1
                            nc.tensor.matmul(
                                out=ps[:], lhsT=g[:, col, :], rhs=ones_sb[:],
                                start=first, stop=last)
                    nc.scalar.copy(out=acc[:, w * P:(w + 1) * P], in_=ps[:])
                    continue
                if ABL == "conly":
                    sg = spool.tile([P, B, P], g_dt)
                    nc.vector.tensor_tensor(
                        out=sg[:, :, :],
                        in0=dstv_sb[:, w * B:(w + 1) * B]
                        .unsqueeze(2).broadcast_to([P, B, P]),
                        in1=iota_sb[:, :].unsqueeze(1).broadcast_to([P, B, P]),
                        op=mybir.AluOpType.is_equal,
                    )
                    nc.vector.tensor_tensor(
                        out=sg[:, :, :],
                        in0=sg[:, :, :],
                        in1=ewv_sb[:, w * B:(w + 1) * B]
                        .unsqueeze(2).broadcast_to([P, B, P]),
                        op=mybir.AluOpType.mult,
                    )
                    ps = pspool.tile([P, P], f32, space="PSUM")
                    for m in range(NCHUNKS):
                        for qq in range(q):
                            first = m == 0 and qq == 0
                            last = m == NCHUNKS - 1 and qq == q - 1
                            nc.tensor.matmul(
                                out=ps[:], lhsT=ones_sb[:],
                                rhs=sg[:, m * q + qq, :],
                                start=first, stop=last)
                    nc.scalar.copy(out=acc[:, w * P:(w + 1) * P], in_=ps[:])
                    continue
                ps = pspool.tile([P, P], f32, space="PSUM")
                for m in range(NCHUNKS):
                    for qq in range(q):
                        cb = w * q + qq
                        t, col = divmod(cb, NB)
                        g = ensure_gather(m, t)
                        first = m == 0 and qq == 0
                        last = m == NCHUNKS - 1 and qq == q - 1
                        nc.tensor.matmul(
                            out=ps[:],
                            lhsT=g[:, col, :],
                            rhs=sg[:, m * q + qq, :],
                            start=first,
                            stop=last,
                        )
                nc.scalar.copy(out=acc[:, w * P:(w + 1) * P], in_=ps[:])

            for cix in range(NOCHUNK):
                ps2 = ps2pool.tile([P, 512], f32, space="PSUM")
                nc.tensor.matmul(
                    out=ps2[:],
                    lhsT=wt_sb[:],
                    rhs=acc[:, cix * 512:(cix + 1) * 512],
                    start=True,
                    stop=True,
                )
                st = stpool.tile([P, 512], f32)
                nc.vector.tensor_scalar(
                    out=st[:],
                    in0=ps2[:],
                    scalar1=bias_sb[:, 0:1],
                    scalar2=None,
                    op0=mybir.AluOpType.add,
                )
                nc.sync.dma_start(out=out_d[:, cix * 512:(cix + 1) * 512], in_=st[:])

        def ensure_gather(m, t):
            if (m, t) in g_tiles:
                return g_tiles[(m, t)]
            nb = min(NB, CSB - t * NB)
            n_idx = nb * P
            g = gpools[m].tile([P, NB, F], g_dt)
            nc.gpsimd.dma_gather(
                out_ap=g[:, :nb, :],
                in_ap=AP(x_h, m * CHUNK * P, [(P, CHUNK), (1, P)]),
                idxs_ap=idx_sb[
                    :, m * (NIDX // 16) + t * NB * 8:
                       m * (NIDX // 16) + t * NB * 8 + n_idx // 16
                ],
                num_idxs=n_idx,
                num_idxs_reg=n_idx,
                elem_size=F,
                single_packet=False,
                queue_num=m,
            )
            g_tiles[(m, t)] = g
            return g

        if REPS > 1:
            with tc.For_i(0, REPS, 1):
                body()
        else:
            body()

    nc.compile()
    _PROGRAM_CACHE[key] = nc
    return nc


def _prepare(x, W, bias, edge_src, edge_dst, edge_weight, idx_keep):
    q = Q
    while True:
        pre = _preprocess(edge_src, edge_dst, edge_weight, idx_keep, q)
        if pre is not None:
            break
        q += 1
    idxbuf, dstv, ewv, metas, W_CAP, NBLK, CSB = pre
    nc = _build_program(W_CAP, q, NBLK, CSB)

    if GDT == "bf16":
        import ml_dtypes
        x = np.ascontiguousarray(
            np.asarray(x, dtype=np.float32).astype(ml_dtypes.bfloat16)
        )
        dstv = dstv.astype(ml_dtypes.bfloat16)
        ewv = ewv.astype(ml_dtypes.bfloat16)
    else:
        x = np.ascontiguousarray(x, dtype=np.float32)
    wt = np.ascontiguousarray(np.asarray(W, dtype=np.float32).T)
    biasv = np.ascontiguousarray(np.asarray(bias, dtype=np.float32).reshape(P, 1))
    in_maps = [
        {
            "x": x,
            "idx": idxbuf[c],
            "dstv": dstv[c],
            "ewv": ewv[c],
            "wt": wt,
            "biasv": biasv,
        }
        for c in range(NC)
    ]
    return nc, in_maps, metas


def _unpack(results, metas):
    out = np.empty((N_NODES, F), np.float32)
    for c in range(NC):
        o = results[c]["out"]  # [P, W_CAP*P], rows = out features
        base = c * NPC
        for w, (s, e) in enumerate(metas[c]):
            out[base + s:base + e, :] = o[:, w * P:w * P + (e - s)].T
    return out


def kernel(x, W, bias, edge_src, edge_dst, edge_weight, idx_keep):
    nc, in_maps, metas = _prepare(
        x, W, bias, edge_src, edge_dst, edge_weight, idx_keep
    )
    res = run_bass_kernel_spmd(nc, in_maps, list(range(NC)))
    return _unpack(res.results, metas)


# --- helpers for test.py (not used by the grading harness) ---------------

def run_traced(x, W, bias, edge_src, edge_dst, edge_weight, idx_keep):
    nc, in_maps, metas = _prepare(
        x, W, bias, edge_src, edge_dst, edge_weight, idx_keep
    )
    res = run_bass_kernel_spmd(nc, in_maps, list(range(NC)), trace=True)
    return _unpack(res.results, metas), res


def run_sim(x, W, bias, edge_src, edge_dst, edge_weight, idx_keep, cores=(0,)):
    from concourse.bass_interp import CoreSim

    nc, in_maps, metas = _prepare(
        x, W, bias, edge_src, edge_dst, edge_weight, idx_keep
    )
    results = []
    for c in cores:
        sim = CoreSim(nc)
        for k, v in in_maps[c].items():
            sim.tensor(k)[:] = v
        sim.simulate()
        results.append({"out": sim.tensor("out").copy()})
    return results, metas, in_maps

